# revision 1
# baseline (speedup 1.0000x reference)
"""Causal self-attention (B=2, T=2048, C=1024, H=16, RoPE) on 8 NeuronCores.

Sharding: core i = (batch b = i//4, head-group g = i%4), 4 heads per core.
Each core computes qkv for its batch restricted to its 4 heads, RoPE,
causal flash attention in a fully transposed layout (no on-device
transposes), and a partial output projection y_partial = O_cat @ Wp[rows].
The host sums the 4 partials per batch and adds b_proj.

Device-side tricks:
  - All matmuls run as float32r (TF32-like, 1 cycle/row at N>=256).
  - x is fed pre-transposed [C, T] by the host; an appended ones-row in
    x^T and a bias-row in W~ fold the qkv bias into the matmul (K=1025).
  - q/k weight columns are permuted on the host to [even dims, odd dims]
    so RoPE becomes two 32-row-block ops: rot = qa*C1 + (Pswap@qa)*C2,
    with Pswap applied on the PE.
  - S^T tiles [k_chunk=128, q_quad=512] are computed directly (lhsT=k^T,
    rhs=q^T), exp on ScalarE with scale=1/8 and NO max subtraction
    (logits are O(5), exp is safe in fp32), causal handled by masking
    diagonal-chunk tiles after exp.
  - V carries an extra all-ones column (weights=0, bias=1) per head, so
    the PV matmul (M=65) yields the softmax denominators in row 64 free.
  - O^T is scaled by 1/s via reciprocal_approx_fast + a K=1 broadcast
    matmul, written into O_cat^T which is directly the lhsT of the
    output projection.
"""

import os
import numpy as np

import concourse.bass as bass
import concourse.mybir as mybir
import concourse.tile as tile
from concourse import bacc
from concourse.bass import ts
from concourse.bass_utils import run_bass_kernel_spmd

F32 = mybir.dt.float32
F32R = mybir.dt.float32r
AF = mybir.ActivationFunctionType

B, T, C = 2, 2048, 1024
NH, D = 16, 64
HPC = 4            # heads per core
NCORES = 8
FQ = HPC * D       # 256 q features per core
FK = HPC * D       # 256 k features per core
FV = HPC * (D + 1)  # 260 v features per core (65th col = ones)
FQK = FQ + FK       # 512
FTOT = FQ + FK + FV  # 772
KC = C // 128      # 8 full contraction chunks (+1 bias row)
QW = 512           # q tile width (quad of 128-blocks)
NQ = T // QW       # 4 q quads
NTB = T // 128     # 16 T blocks


def _build_nc():
    nc = bacc.Bacc(
        "TRN2",
        target_bir_lowering=False,
        debug=False,
        enable_asserts=False,
        num_devices=NCORES,
    )
    xt = nc.dram_tensor("xt", [C, T], F32R, kind="ExternalInput").ap()
    w = nc.dram_tensor("w", [C, FTOT], F32R, kind="ExternalInput").ap()
    bqk = nc.dram_tensor("bqk", [FQK], F32, kind="ExternalInput").ap()
    bv = nc.dram_tensor("bv", [128, FV], F32, kind="ExternalInput").ap()
    wp = nc.dram_tensor("wp", [FQ, C], F32R, kind="ExternalInput").ap()
    c1d = nc.dram_tensor("c1", [128, T], F32, kind="ExternalInput").ap()
    c2d = nc.dram_tensor("c2", [128, T], F32, kind="ExternalInput").ap()
    pmd = nc.dram_tensor("pm", [128, 128], F32R, kind="ExternalInput").ap()
    mskd = nc.dram_tensor("msk", [128, 128], F32, kind="ExternalInput").ap()
    y = nc.dram_tensor("y", [T, C], F32, kind="ExternalOutput").ap()
    dbg = None
    if os.environ.get("KDBG"):
        dbg = {
            "qdbg": nc.dram_tensor("qdbg", [512, T], F32R, kind="ExternalOutput").ap(),
            "vdbg": nc.dram_tensor("vdbg", [128, FV], F32R, kind="ExternalOutput").ap(),
            "odbg": nc.dram_tensor("odbg", [256, T], F32R, kind="ExternalOutput").ap(),
        }

    with tile.TileContext(nc) as tc:
        _body(tc, xt, w, wp, bqk, bv, c1d, c2d, pmd, mskd, y, dbg)
    nc.compile()
    return nc


def _body(tc, xt, w, wp, bqk, bv, c1d, c2d, pmd, mskd, y, dbg=None):
    from contextlib import ExitStack

    nc = tc.nc

    with ExitStack() as ctx:
        # ---- pools that live for the whole kernel ----
        qpool = ctx.enter_context(tc.tile_pool(name="qk", bufs=1))
        psp = ctx.enter_context(tc.tile_pool(name="psp", bufs=1, space="PSUM"))
        vpool = ctx.enter_context(tc.tile_pool(name="v", bufs=1))
        opool = ctx.enter_context(tc.tile_pool(name="ocat", bufs=1))
        wppool = ctx.enter_context(tc.tile_pool(name="wp", bufs=1))
        smal = ctx.enter_context(tc.tile_pool(name="small", bufs=1))

        qa = [qpool.tile([128, T], F32R, name=f"qa{m}", tag=f"qa{m}") for m in range(4)]
        vt = [vpool.tile([128, FV], F32R, name=f"v{i}", tag=f"v{i}") for i in range(NTB)]
        ocat = [opool.tile([128, T], F32R, name=f"oc{i}", tag=f"oc{i}") for i in range(2)]
        wpt = [wppool.tile([128, C], F32R, name=f"wp{i}", tag=f"wp{i}") for i in range(2)]

        # =========== Phase A: qkv^T + v + RoPE ===========
        with ExitStack() as actx:
            xpool = actx.enter_context(tc.tile_pool(name="xt", bufs=1))
            wpool = actx.enter_context(tc.tile_pool(name="w", bufs=1))
            cpool = actx.enter_context(tc.tile_pool(name="ctab", bufs=1))
            spool = actx.enter_context(tc.tile_pool(name="scr", bufs=1))

            xtt = [xpool.tile([128, T], F32R, name=f"x{i}", tag=f"x{i}") for i in range(KC)]
            wtt = [wpool.tile([128, FTOT], F32R, name=f"w{i}", tag=f"w{i}") for i in range(KC)]
            qeng = [nc.sync, nc.scalar, nc.gpsimd]
            qi = 0
            for i in range(KC):
                rs = ts(i, 128)
                qeng[qi % 3].dma_start(wtt[i][:, 0:512], w[rs, 0:512])
                qi += 1
                for hcol in range(2):
                    cs = slice(hcol * 1024, (hcol + 1) * 1024)
                    qeng[qi % 3].dma_start(xtt[i][:, cs], xt[rs, cs])
                    qi += 1
                qeng[qi % 3].dma_start(wtt[i][:, 512:FTOT], w[rs, 512:FTOT])
                qi += 1
            c1t = cpool.tile([128, T], F32, tag="c1")
            c2t = cpool.tile([128, T], F32, tag="c2")
            pmt = cpool.tile([128, 128], F32R, tag="pm")
            nc.sync.dma_start(pmt[:], pmd[:, :])
            nc.scalar.dma_start(c1t[:], c1d[:, :])
            nc.gpsimd.dma_start(c2t[:], c2d[:, :])
            bqk_t = cpool.tile([128, 4], F32, tag="bqk")
            bsrc = bass.AP(bqk.tensor, 0, [[1, 128], [128, 4]])
            nc.sync.dma_start(bqk_t[:, :].unsqueeze(1), bsrc)
            bv_t = cpool.tile([128, FV], F32, tag="bv")
            nc.sync.dma_start(bv_t[:, :], bv[:, :])

            # q^T,k^T: out [feat 512 -> 4 blocks of 128, T]
            def qkt_block(mb):
                for nb in range(4):
                    ps = psp.tile([128, 512], F32, tag="qkv", bufs=4, name="psqkv")
                    for ck in range(KC):
                        nc.tensor.matmul(
                            ps[:, :],
                            lhsT=wtt[ck][:, ts(mb, 128)],
                            rhs=xtt[ck][:, ts(nb, 512)],
                            start=(ck == 0),
                            stop=(ck == KC - 1),
                        )
                    nc.scalar.activation(
                        qa[mb][:, ts(nb, 512)],
                        ps[:, :],
                        AF.Identity,
                        bias=bqk_t[:, mb : mb + 1],
                    )

            # v: out [T -> 16 blocks of 128, 260]
            def v_block(tb):
                psv = psp.tile([128, 512], F32, tag="qkv", bufs=4, name="psqkv")
                for ck in range(KC):
                    nc.tensor.matmul(
                        psv[:, :FV],
                        lhsT=xtt[ck][:, ts(tb, 128)],
                        rhs=wtt[ck][:, FQ + FK : FTOT],
                        start=(ck == 0),
                        stop=(ck == KC - 1),
                    )
                nc.vector.tensor_add(vt[tb][:, :], psv[:, :FV], bv_t[:, :])

            # RoPE on the 4 q/k feature blocks (in place in qa), per
            # 512-col chunk so attention can start on early columns
            def rope_block(mb):
                tcos = spool.tile([128, T], F32, tag="tcos", bufs=2)
                sws = []
                for nb in range(4):
                    nc.vector.tensor_mul(
                        tcos[:, ts(nb, 512)],
                        qa[mb][:, ts(nb, 512)],
                        c1t[:, ts(nb, 512)],
                    )
                    psw = psp.tile([128, 512], F32, tag="qkv", bufs=4, name="psw")
                    nc.tensor.matmul(
                        psw[:, :],
                        lhsT=pmt[:, :],
                        rhs=qa[mb][:, ts(nb, 512)],
                        start=True,
                        stop=True,
                    )
                    sws.append(psw)
                for nb in range(4):
                    nc.vector.tensor_mul(
                        qa[mb][:, ts(nb, 512)], sws[nb][:, :], c2t[:, ts(nb, 512)]
                    )
                    nc.gpsimd.tensor_add(
                        qa[mb][:, ts(nb, 512)],
                        qa[mb][:, ts(nb, 512)],
                        tcos[:, ts(nb, 512)],
                    )
            for mb in (0, 2):
                qkt_block(mb)
                rope_block(mb)
            for mb in (1, 3):
                qkt_block(mb)
                rope_block(mb)
            for tb in range(NTB):
                v_block(tb)

            if dbg is not None:
                for mb in range(4):
                    nc.sync.dma_start(dbg["qdbg"][ts(mb, 128), :], qa[mb][:, :])
                nc.sync.dma_start(dbg["vdbg"][:, :], vt[0][:, :])

        # =========== Phase B: attention + projection ===========
        with ExitStack() as bctx:
            mpool = bctx.enter_context(tc.tile_pool(name="mskp", bufs=1))
            ptpool = bctx.enter_context(tc.tile_pool(name="pt", bufs=1))
            ypool = bctx.enter_context(tc.tile_pool(name="ysb", bufs=1))
            rpool = bctx.enter_context(tc.tile_pool(name="rcp", bufs=1))

            mtri = mpool.tile([128, 128], F32, tag="mtri")
            nc.sync.dma_start(mtri[:], mskd[:, :])
            for i in range(2):
                nc.scalar.dma_start(wpt[i][:], wp[ts(i, 128), :])

            for qq in range(NQ):
                nch = 4 * qq + 4  # causal k chunks of 128
                psos = {}
                scol = rpool.tile([128, 512], F32, tag="scol", bufs=2, name="scol")
                nc.gpsimd.memset(scol[:, :], 1.0)
                for hp in range(2):  # head pair (2*hp, 2*hp+1)
                    qtile = qa[hp]
                    ktile = qa[2 + hp]
                    pso0 = psp.tile([128, 512], F32, tag="qkv", bufs=4, name="pso0")
                    pso1 = psp.tile([128, 512], F32, tag="qkv", bufs=4, name="pso1")
                    psos[(hp, 0)] = pso0
                    psos[(hp, 1)] = pso1
                    for j in range(nch):
                        dj = j - 4 * qq
                        lo = 128 * dj if dj > 0 else 0  # first live q col
                        nj = 512 - lo
                        pss = psp.tile([128, 1024], F32, tag="s", bufs=2, name="pss")
                        nc.tensor.matmul(
                            pss[:, lo:512],
                            lhsT=ktile[0:64, ts(j, 128)],
                            rhs=qtile[0:64, qq * QW + lo : (qq + 1) * QW],
                            start=True,
                            stop=True,
                        )
                        nc.tensor.matmul(
                            pss[:, 512 + lo : 1024],
                            lhsT=ktile[64:128, ts(j, 128)],
                            rhs=qtile[64:128, qq * QW + lo : (qq + 1) * QW],
                            start=True,
                            stop=True,
                            tile_position=(64, 0),
                        )
                        pt = ptpool.tile([128, 1024], F32R, tag="pt", bufs=3, name="pt")
                        pss_v = pss[:, :].rearrange("p (h c) -> p h c", h=2)[:, :, lo:]
                        pt_v = pt[:, :].rearrange("p (h c) -> p h c", h=2)[:, :, lo:]
                        nc.scalar.activation(pt_v, pss_v, AF.Exp, scale=0.125)
                        if dj >= 0:  # diagonal chunk: tri-mask its 128-col block
                            for half in range(2):
                                dcol = half * 512 + lo
                                nc.vector.tensor_mul(
                                    pt[:, dcol : dcol + 128],
                                    pt[:, dcol : dcol + 128],
                                    mtri[:, :],
                                )
                        nc.tensor.matmul(
                            pso0[0:65, lo:512],
                            lhsT=vt[j][:, (2 * hp) * 65 : (2 * hp) * 65 + 65],
                            rhs=pt[:, lo:512],
                            start=(j == 0),
                            stop=(j == nch - 1),
                        )
                        nc.tensor.matmul(
                            pso1[0:65, lo:512],
                            lhsT=vt[j][:, (2 * hp + 1) * 65 : (2 * hp + 1) * 65 + 65],
                            rhs=pt[:, 512 + lo : 1024],
                            start=(j == 0),
                            stop=(j == nch - 1),
                        )
                    for par in range(2):
                        nc.vector.tensor_copy(
                            scol[32 * (2 * hp + par) : 32 * (2 * hp + par) + 1, :],
                            psos[(hp, par)][64:65, :],
                        )
                rinv = rpool.tile([128, 512], F32, tag="rinv", bufs=2, name="rinv")
                nc.vector.reciprocal(rinv[:, :], scol[:, :])
                for hp in range(2):
                    for par in range(2):
                        r = 32 * (2 * hp + par)
                        bp = par * 64
                        rbs = rpool.tile([64, 512], F32, tag="rbs", bufs=4, name="rbs")
                        rsrc = bass.AP(
                            rinv.tensor,
                            rinv[r : r + 1, :].offset,
                            [list(rinv[r : r + 1, :].ap[0]), [0, 64], [1, 512]],
                        )
                        (nc.scalar if par else nc.sync).dma_start(
                            rbs[:, :].unsqueeze(1), rsrc
                        )
                        nc.vector.tensor_mul(
                            ocat[hp][bp : bp + 64, ts(qq, QW)],
                            psos[(hp, par)][0:64, :],
                            rbs[:, :],
                        )

                for qb in range(4 * qq, 4 * qq + 4):
                    qc = qb * 128
                    ysb = ypool.tile([128, C], F32, tag="ysb", bufs=3)
                    psy0 = psp.tile([128, 512], F32, tag="qkv", bufs=4, name="psy0")
                    psy1 = psp.tile([128, 512], F32, tag="qkv", bufs=4, name="psy1")
                    for nb2, psy in ((0, psy0), (1, psy1)):
                        for kk in range(2):
                            nc.tensor.matmul(
                                psy[:, :],
                                lhsT=ocat[kk][:, qc : qc + 128],
                                rhs=wpt[kk][:, ts(nb2, 512)],
                                start=(kk == 0),
                                stop=(kk == 1),
                            )
                    nc.scalar.activation(ysb[:, 0:512], psy0[:, :], AF.Copy)
                    nc.vector.tensor_copy(ysb[:, 512:1024], psy1[:, :])
                    [nc.sync, nc.scalar, nc.gpsimd][qb % 3].dma_start(
                        y[qc : qc + 128, :], ysb[:, :]
                    )

            if dbg is not None:
                for i in range(2):
                    nc.sync.dma_start(dbg["odbg"][ts(i, 128), :], ocat[i][:, :])


_PERM = np.concatenate([np.arange(0, 64, 2), np.arange(1, 64, 2)])


def _host_inputs(x, W_attn, b_attn, W_proj):
    """Build the 8 per-core input dicts."""
    x = np.asarray(x, np.float32)
    W_attn = np.asarray(W_attn, np.float32)
    b_attn = np.asarray(b_attn, np.float32)
    W_proj = np.asarray(W_proj, np.float32)

    half = D // 2
    inv = 1.0 / (10000.0 ** (np.arange(half, dtype=np.float64) * 2.0 / D))
    emb = np.outer(np.arange(T, dtype=np.float64), inv)  # [T, 32]
    cosT = np.cos(emb).T.astype(np.float32)  # [32, T]
    sinT = np.sin(emb).T.astype(np.float32)
    c1 = np.ascontiguousarray(np.tile(cosT, (4, 1)))
    c2 = np.ascontiguousarray(
        np.concatenate([-sinT, sinT, -sinT, sinT], axis=0)
    )
    pm = np.zeros((128, 128), np.float32)
    ii = np.arange(128)
    swp = np.where((ii % 64) < 32, ii + 32, ii - 32)
    pm[swp, ii] = 1.0
    kk = np.arange(128)[:, None]
    qv = np.arange(128)[None, :]
    msk = (qv >= kk).astype(np.float32)  # [128,128] lower-left-of-diag masked

    Wq, Wk, Wv = W_attn[:, :C], W_attn[:, C : 2 * C], W_attn[:, 2 * C :]
    bq, bk, bv = b_attn[:C], b_attn[C : 2 * C], b_attn[2 * C :]

    xts = [np.ascontiguousarray(x[b].T) for b in range(B)]

    in_maps = []
    for core in range(NCORES):
        b, g = core // 4, core % 4
        heads = range(g * HPC, (g + 1) * HPC)
        wq_g = np.concatenate([Wq[:, h * D + _PERM] for h in heads], 1)
        wk_g = np.concatenate([Wk[:, h * D + _PERM] for h in heads], 1)
        wv_g = np.concatenate(
            [
                np.concatenate(
                    [Wv[:, h * D : (h + 1) * D], np.zeros((C, 1), np.float32)], 1
                )
                for h in heads
            ],
            1,
        )
        bq_g = np.concatenate([bq[h * D + _PERM] for h in heads])
        bk_g = np.concatenate([bk[h * D + _PERM] for h in heads])
        bv_g = np.concatenate(
            [
                np.concatenate([bv[h * D : (h + 1) * D], np.ones(1, np.float32)])
                for h in heads
            ]
        )
        w_full = np.ascontiguousarray(
            np.concatenate([wq_g, wk_g, wv_g], 1)
        )  # [C, 772]
        bqk_g = np.ascontiguousarray(np.concatenate([bq_g, bk_g]))
        bv_full = np.ascontiguousarray(np.tile(bv_g[None, :], (128, 1)))
        wp_g = np.ascontiguousarray(W_proj[g * FQ : (g + 1) * FQ, :])
        in_maps.append(
            {
                "xt": xts[b],
                "w": w_full,
                "wp": wp_g,
                "bqk": bqk_g,
                "bv": bv_full,
                "c1": c1,
                "c2": c2,
                "pm": pm,
                "msk": msk,
            }
        )
    return in_maps


_NC_CACHE = {}


def _get_nc():
    if "nc" not in _NC_CACHE:
        _NC_CACHE["nc"] = _build_nc()
    return _NC_CACHE["nc"]


def _run(inputs, trace=False):
    nc = _get_nc()
    in_maps = _host_inputs(
        inputs["x"], inputs["W_attn"], inputs["b_attn"], inputs["W_proj"]
    )
    res = run_bass_kernel_spmd(nc, in_maps, core_ids=list(range(NCORES)), trace=trace)
    b_proj = np.asarray(inputs["b_proj"], np.float32)
    out = np.zeros((B, T, C), np.float32)
    for b in range(B):
        acc = np.zeros((T, C), np.float32)
        for g in range(4):
            acc += res.results[b * 4 + g]["y"]
        out[b] = acc + b_proj[None, :]
    return out, res


def _bench(inputs, iters=10):
    """Time the NEFF on HW: chain `iters` dependent executions inside one
    jitted program (outputs feed the next call's donated-zero slots), for
    two different chain lengths, and report the marginal per-exec time.
    Returns (out, per_exec_ns)."""
    import time as _time

    import jax
    from jax.sharding import Mesh, NamedSharding, PartitionSpec
    from jax.experimental.shard_map import shard_map

    from concourse import bass2jax

    nc = _get_nc()
    in_maps = _host_inputs(
        inputs["x"], inputs["W_attn"], inputs["b_attn"], inputs["W_proj"]
    )
    bass2jax.install_neuronx_cc_hook()

    partition_name = nc.partition_id_tensor.name if nc.partition_id_tensor else None
    in_names, out_names, out_avals, zero_outs = [], [], [], []
    for alloc in nc.m.functions[0].allocations:
        if not isinstance(alloc, mybir.MemoryLocationSet):
            continue
        name = alloc.memorylocations[0].name
        if alloc.kind == "ExternalInput":
            if name != partition_name:
                in_names.append(name)
        elif alloc.kind == "ExternalOutput":
            out_names.append(name)
            shape = tuple(alloc.tensor_shape)
            dtype = mybir.dt.np(alloc.dtype)
            out_avals.append(jax.core.ShapedArray(shape, dtype))
            zero_outs.append(np.zeros(shape, dtype))
    n_params = len(in_names)
    all_in_names = in_names + out_names
    if partition_name is not None:
        all_in_names = all_in_names + [partition_name]

    def make_body(m):
        def _body(*args):
            params = list(args[:n_params])
            carry = tuple(args[n_params:])
            extra = []
            if partition_name is not None:
                extra = [bass2jax.partition_id_tensor()]
            for _ in range(m):
                carry = _bass_exec_bind(params, carry, extra)
            return carry

        def _bass_exec_bind(params, carry, extra):
            outs = bass2jax._bass_exec_p.bind(
                *params,
                *carry,
                *extra,
                out_avals=tuple(out_avals),
                in_names=tuple(all_in_names),
                out_names=tuple(out_names),
                lowering_input_output_aliases=(),
                sim_require_finite=True,
                sim_require_nnan=True,
                nc=nc,
            )
            return tuple(outs)

        return _body

    devices = jax.devices()[:NCORES]
    mesh = Mesh(np.asarray(devices), ("core",))
    spec = PartitionSpec("core")
    n_args = n_params + len(out_names)
    per_core = [[np.asarray(m[name]) for name in in_names] for m in in_maps]
    concat_in = [
        np.concatenate([per_core[c][i] for c in range(NCORES)], axis=0)
        for i in range(n_params)
    ]
    concat_zeros = [
        np.zeros((NCORES * z.shape[0], *z.shape[1:]), z.dtype) for z in zero_outs
    ]
    sh = NamedSharding(mesh, spec)
    dev_args = [jax.device_put(a, sh) for a in concat_in + concat_zeros]

    fn = jax.jit(
        shard_map(
            make_body(1), mesh=mesh, in_specs=(spec,) * n_args,
            out_specs=(spec,) * len(out_names), check_rep=False,
        ),
        keep_unused=True,
    )
    outs = fn(*dev_args)  # warmup + compile
    jax.block_until_ready(outs)

    def timed(n, reps=5):
        best = float("inf")
        out_local = None
        for _ in range(reps):
            t0 = _time.perf_counter()
            for _i in range(n):
                out_local = fn(*dev_args)
            jax.block_until_ready(out_local)
            best = min(best, _time.perf_counter() - t0)
        return best, out_local

    n_low, n_high = 2, 2 + iters
    t_low, _ = timed(n_low)
    t_high, outs = timed(n_high)
    per_exec_ns = (t_high - t_low) / (n_high - n_low) * 1e9

    res = [
        {
            name: np.asarray(outs[i]).reshape(NCORES, *out_avals[i].shape)[c]
            for i, name in enumerate(out_names)
        }
        for c in range(NCORES)
    ]
    b_proj = np.asarray(inputs["b_proj"], np.float32)
    out = np.zeros((B, T, C), np.float32)
    for b in range(B):
        acc = np.zeros((T, C), np.float32)
        for g in range(4):
            acc += res[b * 4 + g]["y"]
        out[b] = acc + b_proj[None, :]
    return out, per_exec_ns


def kernel(x, W_attn, b_attn, W_proj, b_proj):
    out, _ = _run(
        {
            "x": x,
            "W_attn": W_attn,
            "b_attn": b_attn,
            "W_proj": W_proj,
            "b_proj": b_proj,
        },
        trace=False,
    )
    return out

